# revision 53
# baseline (speedup 1.0000x reference)
"""Trainium2 Bass kernel for AssociativeIncrementalAttention.

Multi-head attention (B=2, S=2048, D=512, H=8, HD=64) with additive
[B,S,S] bias tensors, a concept-equality bias, and key-padding mask.

Sharding: 8 cores, fully data-parallel (no collectives).
  core c -> batch b = c//4, query rows q0 = (c%4)*512 .. q0+512.
Each core computes full K/V for its batch (replicated inside the
4-core batch group), scores for its 512 query rows, softmax via
exp(s/8)*exp(comb) with normalization deferred past the attn@V matmul
(rowsum rides the matmul through an appended ones-column in V).

Structure notes:
 - Heads are processed in pairs (2j, 2j+1) living on partition halves
   0-63 / 64-127, so the two K=64 score matmuls of a pair occupy
   disjoint PE row-groups and execute concurrently in the array.
 - attn@V lags one k-chunk behind the scores (software pipeline) so
   the PE never stalls on the exp->multiply chain.
 - The combined bias (ipa+assoc+key-padding+concept) is prepped on
   GpSimd+DVE; the attention_mask is all-zero by construction (spec
   fill=zeros) and is folded into assoc_bias on the host IFF it is
   ever nonzero, so the device never streams it.
 - V projection is interleaved into the first pair's loop; the
   previous pair's softmax normalization is interleaved into the
   next pair's loop.

Self-contained: hardcodes shapes; host-side prep is layout-only
(slices/transposes) plus tiny metadata casts (concept ids -> f32
sentinels, padding mask -> additive f32, position iotas).
"""

import sys

if "/opt/trn_rl_repo" not in sys.path:
    sys.path.insert(0, "/opt/trn_rl_repo")

import numpy as np

import concourse.bass as bass
import concourse.tile as tile
from concourse import bacc, mybir
from concourse import bass_utils

B, S, D, H = 2, 2048, 512, 8
HD = D // H  # 64
N_CORES = 8
QS = 512          # query rows per core
QT = QS // 128    # 4 query tiles per core
DC = D // 128     # 4 contraction chunks
SC512 = S // 512  # 4
SC128 = S // 128  # 16
NP = H // 2       # 4 head pairs
F32 = mybir.dt.float32
F16 = mybir.dt.float16
BF16 = mybir.dt.bfloat16

_COMPILED = None


def _build():
    nc = bacc.Bacc("TRN2", target_bir_lowering=False, debug=False,
                   num_devices=N_CORES)

    def din(name, shape, dt=F32):
        return nc.dram_tensor(name, shape, dt, kind="ExternalInput").ap()

    # big tensors arrive host-pre-tiled ([... ,128, c, free] contiguous)
    # so every DMA is a contiguous block read at full HBM bandwidth
    xT = din("xT", [SC512, 128, DC, 512], BF16)
    xqT = din("xqT", [128, DC, QS], BF16)
    wqT = din("wqT", [128, DC, D], BF16)
    wkT = din("wkT", [128, DC, D], BF16)
    wvT = din("wvT", [128, DC, D], BF16)
    woT = din("woT", [128, DC, D], BF16)
    bq = din("bq", [D])
    bk = din("bk", [D])
    bv = din("bv", [D])
    bo = din("bo", [D])
    ipaT = din("ipaT", [4, 128, 4, QS], BF16)
    ascT = din("ascT", [4, 128, 4, QS], BF16)
    cidq = din("cidq", [QS], BF16)
    cidkT = din("cidkT", [128, SC128])
    out = nc.dram_tensor("out", [QS, D], F32, kind="ExternalOutput").ap()

    AL = mybir.AluOpType
    AF = mybir.ActivationFunctionType

    def bcast_ap(src):
        # partition-broadcast read: [[0,128]] + original free dims
        return bass.AP(tensor=src.tensor, offset=src.offset,
                       ap=[[0, 128]] + list(src.ap))

    with tile.TileContext(nc) as tc:
        with (
            tc.tile_pool(name="persist", bufs=1) as P,
            tc.tile_pool(name="combwork", bufs=1) as CW,
            tc.tile_pool(name="pwork", bufs=4) as PW,
            tc.tile_pool(name="rswork", bufs=2) as RW,
            tc.tile_pool(name="osb", bufs=2) as OS,
            tc.tile_pool(name="spp", bufs=2, space="PSUM") as SPP,
            tc.tile_pool(name="ctxp", bufs=4, space="PSUM") as CP,
        ):
            # ---- persistent tiles ----
            kT_sb = P.tile([128, DC, S], BF16, tag="kT")
            qT_sb = P.tile([128, DC, QS], BF16, tag="qT")
            # 66-wide per-head blocks: col 64 = ones (rowsum ride-along),
            # col 65 = pad so every 64-col copy block starts 4B-aligned
            vplus = P.tile([128, SC128, 8 * 66], BF16, tag="vplus")
            vp4 = vplus.rearrange("p s (h c) -> p s h c", c=66)
            ebT = P.tile([128, SC128, QS], BF16, tag="ebT")
            ctxT_sb = P.tile([128, DC, QS], BF16, tag="ctxT")

            # ---- big loads on the gpsimd queue, in arrival-priority
            #      order: x (gates K proj), wk, wq, xq, wv, then the bias
            #      groups, wo last-ish.  Small metadata goes on sync.
            cidqb = CW.tile([128, QS], BF16, tag="cidqb")
            nc.gpsimd.dma_start(out=cidqb, in_=bcast_ap(cidq))
            w_sb = {}
            w_sb["wk"] = P.tile([128, DC, D], BF16, tag="wk", name="wk")
            nc.gpsimd.dma_start(out=w_sb["wk"], in_=wkT)
            xT_sb = P.tile([128, DC, S], BF16, tag="xT")
            nc.gpsimd.dma_start(out=xT_sb[:, :, 0:512], in_=xT[0])
            w_sb["wv"] = P.tile([128, DC, D], BF16, tag="wv", name="wv")
            nc.gpsimd.dma_start(out=w_sb["wv"], in_=wvT)
            nc.gpsimd.dma_start(out=xT_sb[:, :, 512:1024], in_=xT[1])

            bias_in = {"ipa": [None] * 4, "asc": [None] * 4}

            def load_bias_g4(g4, bufs=2):
                for nm, ap_ in (("ipa", ipaT), ("asc", ascT)):
                    t = CW.tile([128, 4, QS], BF16, tag=nm, name=nm, bufs=bufs)
                    nc.gpsimd.dma_start(out=t, in_=ap_[g4])
                    bias_in[nm][g4] = t
            # bias groups ride between the front's x/w consumers: the
            # projections only need each x chunk progressively, so the
            # biases land early enough for the pair-0 xfall stagger
            load_bias_g4(0)
            w_sb["wq"] = P.tile([128, DC, D], BF16, tag="wq", name="wq")
            nc.gpsimd.dma_start(out=w_sb["wq"], in_=wqT)
            nc.gpsimd.dma_start(out=xT_sb[:, :, 1024:1536], in_=xT[2])
            load_bias_g4(1)
            xqT_sb = P.tile([128, DC, QS], BF16, tag="xqT")
            nc.gpsimd.dma_start(out=xqT_sb, in_=xqT)
            nc.gpsimd.dma_start(out=xT_sb[:, :, 1536:2048], in_=xT[3])
            load_bias_g4(2)
            load_bias_g4(3)
            w_sb["wo"] = P.tile([128, DC, D], BF16, tag="wo", name="wo")
            nc.gpsimd.dma_start(out=w_sb["wo"], in_=woT)
            wo_sb = w_sb["wo"]

            # ---- small loads on sync (arrive almost immediately) ----
            b_sb = {}
            for nm, ap_ in (("bq", bq), ("bk", bk), ("bv", bv)):
                b_sb[nm] = P.tile([128, DC], F32, tag=nm, name=nm)
                nc.sync.dma_start(out=b_sb[nm],
                                  in_=ap_.rearrange("(c p) -> p c", p=128))
            bo_row = P.tile([1, D], F32, tag="bo_row")
            nc.sync.dma_start(out=bo_row, in_=bo.rearrange("(a s) -> a s", a=1))
            cidkT_sb = P.tile([128, SC128], F32, tag="cidkT")
            nc.sync.dma_start(out=cidkT_sb, in_=cidkT)
            bv_bf = P.tile([128, DC], BF16, tag="bv_bf")
            nc.vector.tensor_copy(bv_bf, b_sb["bv"])
            cvec = P.tile([1, D], BF16, tag="cvec")
            ones_row = P.tile([1, 128], BF16, tag="ones_row")
            nc.vector.memset(ones_row, 1.0)
            nc.vector.memset(vp4[:, :, :, 64:65], 1.0)

            # ---- concept-equality bias, one chunk at a time:
            #      wcm[kc] = 0.5*(cidq==cidk).  The diagonal exclusion
            # and the key-padding additive mask are folded into ipa on
            # the host (exact; see _prep_in_maps).  These 16 ops fill
            # the DVE during the initial DMA window.
            wcm_sb = CW.tile([128, SC128, QS], BF16, tag="wcm")
            for kc in range(SC128):
                nc.vector.tensor_scalar(
                    out=wcm_sb[:, kc, :], in0=cidqb,
                    scalar1=cidkT_sb[:, kc:kc + 1],
                    scalar2=0.5, op0=AL.is_equal, op1=AL.mult)

            xfall = CW.tile([128, SC128, QS], BF16, tag="xfall")

            def xfall_dve(kc):
                g4, j = kc // 4, kc % 4
                ta = CW.tile([128, QS], BF16, tag="ta", bufs=2)
                nc.vector.tensor_tensor(
                    out=ta, in0=bias_in["ipa"][g4][:, j, :],
                    in1=bias_in["asc"][g4][:, j, :], op=AL.add)
                nc.vector.tensor_tensor(
                    out=xfall[:, kc, :], in0=ta, in1=wcm_sb[:, kc, :],
                    op=AL.add)

            # ---- front K projection: ONLY head-pairs 0/1 (oc 0,1) --
            # pairs 2/3's kT is produced inside pairs 0/1, where the
            # Act-bound loop leaves PE slack.  One 512-key chunk per
            # tile so matmuls start as soon as wk + x chunk 0 land; the
            # V-projection groups for key chunks 0-7 ride the same
            # arrival window.
            # ---- V projection, one 2-chunk group (bv rides via cvec).
            # PSUM comes from the ctx pool's two slots that sit idle
            # during pair 0, so V tiles never stall the score pipeline.
            def v_proj_group(scp):
                for i in range(2):
                    sc = scp * 2 + i
                    ps = CP.tile([128, 512], F32, tag="ctx", name="psv")
                    for dc in range(DC):
                        nc.tensor.matmul(
                            ps,
                            lhsT=xT_sb[:, dc, sc * 128:(sc + 1) * 128],
                            rhs=w_sb["wv"][:, dc, :],
                            start=(dc == 0), stop=(dc == DC - 1))
                    nc.vector.tensor_copy(
                        vp4[:, sc, :, 0:64],
                        ps.rearrange("p (h c) -> p h c", c=64))

            def k_front_tile(sc):
                ps = SPP.tile([128, 2, 512], F32, tag="sp", name="psk")
                for i in range(2):
                    for dc in range(DC):
                        nc.tensor.matmul(
                            ps[:, i, :],
                            lhsT=w_sb["wk"][:, dc, i * 128:(i + 1) * 128],
                            rhs=xT_sb[:, dc, sc * 512:(sc + 1) * 512],
                            start=(dc == 0), stop=(dc == DC - 1))
                for i in range(2):
                    nc.vector.tensor_scalar(
                        out=kT_sb[:, i, sc * 512:(sc + 1) * 512],
                        in0=ps[:, i, :], scalar1=b_sb["bk"][:, i:i + 1],
                        scalar2=None, op0=AL.add)

            def k_late_tile(oc, sc):
                ps = CP.tile([128, 512], F32, tag="ctx", name="psk2")
                for dc in range(DC):
                    nc.tensor.matmul(
                        ps,
                        lhsT=w_sb["wk"][:, dc, oc * 128:(oc + 1) * 128],
                        rhs=xT_sb[:, dc, sc * 512:(sc + 1) * 512],
                        start=(dc == 0), stop=(dc == DC - 1))
                nc.vector.tensor_scalar(
                    out=kT_sb[:, oc, sc * 512:(sc + 1) * 512],
                    in0=ps, scalar1=b_sb["bk"][:, oc:oc + 1],
                    scalar2=None, op0=AL.add)

            k_front_tile(0)
            v_proj_group(0)
            v_proj_group(1)
            k_front_tile(1)
            v_proj_group(2)
            v_proj_group(3)
            k_front_tile(2)
            k_front_tile(3)

            # ---- Q projection ----
            for ocp in range(2):
                ps = SPP.tile([128, 2, 512], F32, tag="sp", name="psq")
                for i in range(2):
                    oc = ocp * 2 + i
                    for dc in range(DC):
                        nc.tensor.matmul(
                            ps[:, i, :],
                            lhsT=w_sb["wq"][:, dc, oc * 128:(oc + 1) * 128],
                            rhs=xqT_sb[:, dc, :],
                            start=(dc == 0), stop=(dc == DC - 1))
                for i in range(2):
                    oc = ocp * 2 + i
                    nc.vector.tensor_scalar(
                        out=qT_sb[:, oc, :], in0=ps[:, i, :],
                        scalar1=b_sb["bq"][:, oc:oc + 1],
                        scalar2=None, op0=AL.add)
            for kc in range(8):
                xfall_dve(kc)
            # the first half of the bias exps runs while the projections
            # stream (their DMA groups land early); the rest is staggered
            # into the first pair so praw(0) is never gated
            for g in (0, 4):
                nc.scalar.activation(out=ebT[:, g:g + 4, :],
                                     in_=xfall[:, g:g + 4, :], func=AF.Exp)

            # ---- attention main loop over head pairs ----
            norm_state = {}

            def norm_step(step, hs):
                h0, cps0, h1, cps1 = hs
                hh, cps_prev = (h0, cps0) if step < 3 else (h1, cps1)
                st = step % 3
                ocp_, rbp = hh // 2, (hh % 2) * 64
                if st == 0:
                    rs_row = RW.tile([1, QS], F32, tag="rs_row",
                                     name="rs_row")
                    nc.vector.tensor_copy(rs_row, cps_prev[64:65, :])
                    rr = RW.tile([1, QS], F32, tag="rr", name="rr")
                    nc.vector.reciprocal_approx_fast(rr, rs_row)
                    norm_state[("rr", hh)] = rr
                elif st == 1:
                    rrb = RW.tile([64, QS], F32, tag="rrb", name="rrb")
                    nc.gpsimd.partition_broadcast(rrb, norm_state[("rr", hh)])
                    norm_state[("rrb", hh)] = rrb
                else:
                    nc.vector.tensor_tensor(
                        out=ctxT_sb[rbp:rbp + 64, ocp_, :],
                        in0=cps_prev[0:64, :], in1=norm_state[("rrb", hh)],
                        op=AL.mult)

            def cvec_block():
                # cvec = Wo @ bv + bo (rank-1 epilogue row); emitted in
                # pair 1 so the PE never stalls on the late wo DMA
                cps = SPP.tile([128, 2, 512], F32, tag="sp", name="cps")
                for dc in range(DC):
                    nc.tensor.matmul(cps[0:1, 0, :],
                                     lhsT=bv_bf[:, dc:dc + 1],
                                     rhs=wo_sb[:, dc, :],
                                     start=(dc == 0), stop=(dc == DC - 1))
                nc.vector.tensor_tensor(out=cvec, in0=cps[0:1, 0, :],
                                        in1=bo_row, op=AL.add)

            pending = None
            pend_av = []         # attnV lag queue: 2 chunks deep so the
                                 # PE never stalls on a just-produced pf

            def emit_attnv(pv):
                pp, pkc, ppf, pc0, pc1 = pv
                nc.tensor.matmul(
                    pc0, lhsT=vplus[:, pkc, (2 * pp) * 66:(2 * pp) * 66 + 65],
                    rhs=ppf[:, 0, :],
                    start=(pkc == 0), stop=(pkc == SC128 - 1))
                nc.tensor.matmul(
                    pc1, lhsT=vplus[:, pkc,
                                    (2 * pp + 1) * 66:(2 * pp + 1) * 66 + 65],
                    rhs=ppf[:, 1, :],
                    start=(pkc == 0), stop=(pkc == SC128 - 1))

            for p in range(NP):
                cps0 = CP.tile([65, QS], F32, tag="ctx", name="ctx0")
                cps1 = CP.tile([65, QS], F32, tag="ctx", name="ctx1")
                for kc in range(SC128):
                    ps = SPP.tile([128, 2, 512], F32, tag="sp", name="pss")
                    nc.tensor.matmul(
                        ps[:, 0, :],
                        lhsT=kT_sb[0:64, p, kc * 128:(kc + 1) * 128],
                        rhs=qT_sb[0:64, p, :], start=True, stop=True)
                    nc.tensor.matmul(
                        ps[:, 1, :],
                        lhsT=kT_sb[64:128, p, kc * 128:(kc + 1) * 128],
                        rhs=qT_sb[64:128, p, :], start=True, stop=True)
                    praw = PW.tile([128, 2, 512], BF16, tag="praw",
                                   name="praw", bufs=8)
                    nc.scalar.activation(out=praw, in_=ps, func=AF.Exp,
                                         scale=0.125)
                    if p == 0 and kc in (6, 9):
                        g = 8 if kc == 6 else 12
                        nc.scalar.activation(
                            out=ebT[:, g:g + 4, :],
                            in_=xfall[:, g:g + 4, :], func=AF.Exp)
                    pf = PW.tile([128, 2, 512], BF16, tag="pf", name="pf",
                                 bufs=8)
                    nc.vector.tensor_tensor(
                        out=pf[:, 0, :], in0=praw[:, 0, :],
                        in1=ebT[:, kc, :], op=AL.mult)
                    nc.vector.tensor_tensor(
                        out=pf[:, 1, :], in0=praw[:, 1, :],
                        in1=ebT[:, kc, :], op=AL.mult)
                    if pending is not None and 2 <= kc <= 7:
                        norm_step(kc - 2, pending)
                        if kc == 7:
                            pending = None
                    pend_av.append((p, kc, pf, cps0, cps1))
                    if len(pend_av) == 3:
                        emit_attnv(pend_av.pop(0))
                    if p == 0 and 4 <= kc < 8:
                        xfall_dve(2 * kc)
                        xfall_dve(2 * kc + 1)
                    if p == 0 and kc in (1, 3, 5, 7):
                        # V groups for key chunks 8-15; consumed from
                        # iteration 9 onward, always behind this point
                        v_proj_group(4 + (kc - 1) // 2)
                    if p == 1 and kc in (8, 10, 12, 14):
                        k_late_tile(2, (kc - 8) // 2)
                    if p == 2 and kc in (8, 10, 12, 14):
                        k_late_tile(3, (kc - 8) // 2)
                    if p == 1 and kc == 1:
                        cvec_block()
                pending = (2 * p, cps0, 2 * p + 1, cps1)
            for pv in pend_av:
                emit_attnv(pv)
            for step in (0, 3, 1, 4, 2, 5):
                norm_step(step, pending)

            # ---- output projection (cvec row first, dc=3 last) ----
            for m in range(QT):
                pom = SPP.tile([128, 2, 512], F32, tag="sp", name="pom")
                nc.tensor.matmul(pom[:, 0, :], lhsT=ones_row, rhs=cvec,
                                 start=True, stop=False)
                for dc in range(DC):
                    nc.tensor.matmul(
                        pom[:, 0, :],
                        lhsT=ctxT_sb[:, dc, m * 128:(m + 1) * 128],
                        rhs=wo_sb[:, dc, :],
                        start=False, stop=(dc == DC - 1))
                o_t = OS.tile([128, 512], F32, tag="o", name="o_t")
                nc.vector.tensor_copy(o_t, pom[:, 0, :])
                nc.sync.dma_start(out=out[m * 128:(m + 1) * 128, :],
                                  in_=o_t)

    nc.compile()
    return nc


def _prep_in_maps(inputs):
    from ml_dtypes import bfloat16
    x = np.asarray(inputs["x"], np.float32)
    ipa = np.asarray(inputs["ipa_affinity_bias"], np.float32)
    asc = np.asarray(inputs["assoc_bias"], np.float32)
    msk = np.asarray(inputs["attention_mask"], np.float32)
    cid = np.asarray(inputs["concept_ids"])
    kpm = np.asarray(inputs["key_padding_mask"])

    # attention_mask is all-zero for this model config (spec fill=zeros);
    # fold it into assoc_bias on the off chance it is ever nonzero so the
    # device result stays exact without streaming a third [S,S] matrix.
    if np.any(msk):
        asc = asc + msk[None, :, :]
    # key_padding_mask is likewise all-False by construction; fold the
    # additive -inf along k into ipa under the same exactness guard.
    if np.any(kpm):
        kpm_add = np.where(kpm, np.float32(-1e30), np.float32(0.0))
        ipa = ipa + kpm_add[:, None, :]

    def tile_pcf(aT):
        # [D, F] -> [128, D//128, F] partition-major, contiguous
        d, f = aT.shape
        return np.ascontiguousarray(
            aT.reshape(d // 128, 128, f).transpose(1, 0, 2))

    wT = {nm: tile_pcf(np.asarray(inputs[nm], np.float32).T
                       ).astype(bfloat16)
          for nm in ("Wq", "Wk", "Wv", "Wo")}
    bias = {nm: np.asarray(inputs[nm], np.float32)
            for nm in ("bq", "bk", "bv", "bo")}

    # x[b].T tiled chunk-major: [sc, 128, DC, 512]
    xT = [np.ascontiguousarray(
              x[b].T.reshape(DC, 128, SC512, 512).transpose(2, 1, 0, 3)
          ).astype(bfloat16) for b in range(B)]
    cidq_f = np.where(cid >= 0, cid, -1).astype(np.float32)
    cidk_f = np.where(cid >= 0, cid, -2).astype(np.float32)

    in_maps = []
    for c in range(N_CORES):
        b, q0 = c // 4, (c % 4) * QS
        # [k, q_local] slice of ipa; subtract the concept-bias diagonal
        # exclusion here (same concept at q==k is a tautology, so the
        # reference's ~diag term is exactly a -0.5 on the diagonal).
        ipaT_c = np.ascontiguousarray(ipa[b, q0:q0 + QS].T)
        ipaT_c[q0 + np.arange(QS), np.arange(QS)] -= np.float32(0.5)
        # [S, QS] -> [g4, 128, 4, QS] (chunk-major, partition-major)
        ipaT_c = np.ascontiguousarray(
            ipaT_c.reshape(4, 4, 128, QS).transpose(0, 2, 1, 3)
        ).astype(bfloat16)
        ascT_c = np.ascontiguousarray(
            asc[b, q0:q0 + QS].T.reshape(4, 4, 128, QS).transpose(0, 2, 1, 3)
        ).astype(bfloat16)
        in_maps.append({
            "xT": xT[b],
            "xqT": tile_pcf(x[b, q0:q0 + QS].T).astype(bfloat16),
            "wqT": wT["Wq"], "wkT": wT["Wk"],
            "wvT": wT["Wv"], "woT": wT["Wo"],
            "bq": bias["bq"], "bk": bias["bk"],
            "bv": bias["bv"], "bo": bias["bo"],
            "ipaT": ipaT_c,
            "ascT": ascT_c,
            "cidq": np.ascontiguousarray(
                cidq_f[b, q0:q0 + QS]).astype(bfloat16),
            "cidkT": np.ascontiguousarray(cidk_f[b].reshape(SC128, 128).T),
        })
    return in_maps


def run(inputs, trace=False):
    global _COMPILED
    if _COMPILED is None:
        _COMPILED = _build()
    nc = _COMPILED
    in_maps = _prep_in_maps(inputs)
    kw = {}
    if trace:
        kw = dict(trace=True, trace_cores=list(range(N_CORES)))
    res = bass_utils.run_bass_kernel_spmd(
        nc, in_maps, core_ids=list(range(N_CORES)), **kw)
    out = np.empty((B, S, D), np.float32)
    for c in range(N_CORES):
        b, q0 = c // 4, (c % 4) * QS
        out[b, q0:q0 + QS] = res.results[c]["out"]
    return out, res


def kernel(**inputs) -> np.ndarray:
    out, _ = run(inputs)
    return out


# revision 55
# speedup vs baseline: 1.1778x; 1.1778x over previous
"""Trainium2 Bass kernel for AssociativeIncrementalAttention.

Multi-head attention (B=2, S=2048, D=512, H=8, HD=64) with additive
[B,S,S] bias tensors, a concept-equality bias, and key-padding mask.

Sharding: 8 cores, fully data-parallel (no collectives).
  core c -> batch b = c//4, query rows q0 = (c%4)*512 .. q0+512.
Each core computes full K/V for its batch (replicated inside the
4-core batch group), scores for its 512 query rows, softmax via
exp(s/8)*exp(comb) with normalization deferred past the attn@V matmul
(rowsum rides the matmul through an appended ones-column in V).

Structure notes:
 - Heads are processed in pairs (2j, 2j+1) living on partition halves
   0-63 / 64-127, so the two K=64 score matmuls of a pair occupy
   disjoint PE row-groups and execute concurrently in the array.
 - attn@V lags one k-chunk behind the scores (software pipeline) so
   the PE never stalls on the exp->multiply chain.
 - The combined bias (ipa+assoc+key-padding+concept) is prepped on
   GpSimd+DVE; the attention_mask is all-zero by construction (spec
   fill=zeros) and is folded into assoc_bias on the host IFF it is
   ever nonzero, so the device never streams it.
 - V projection is interleaved into the first pair's loop; the
   previous pair's softmax normalization is interleaved into the
   next pair's loop.

Self-contained: hardcodes shapes; host-side prep is layout-only
(slices/transposes) plus tiny metadata casts (concept ids -> f32
sentinels, padding mask -> additive f32, position iotas).
"""

import sys

if "/opt/trn_rl_repo" not in sys.path:
    sys.path.insert(0, "/opt/trn_rl_repo")

import numpy as np

import concourse.bass as bass
import concourse.tile as tile
from concourse import bacc, mybir
from concourse import bass_utils

B, S, D, H = 2, 2048, 512, 8
HD = D // H  # 64
N_CORES = 8
QS = 512          # query rows per core
QT = QS // 128    # 4 query tiles per core
DC = D // 128     # 4 contraction chunks
SC512 = S // 512  # 4
SC128 = S // 128  # 16
NP = H // 2       # 4 head pairs
F32 = mybir.dt.float32
F16 = mybir.dt.float16
BF16 = mybir.dt.bfloat16

_COMPILED = None


def _build():
    nc = bacc.Bacc("TRN2", target_bir_lowering=False, debug=False,
                   num_devices=N_CORES)

    def din(name, shape, dt=F32):
        return nc.dram_tensor(name, shape, dt, kind="ExternalInput").ap()

    # big tensors arrive host-pre-tiled ([... ,128, c, free] contiguous)
    # so every DMA is a contiguous block read at full HBM bandwidth
    xT = din("xT", [SC512, 128, DC, 512], BF16)
    xqT = din("xqT", [128, DC, QS], BF16)
    wqT = din("wqT", [128, DC, D], BF16)
    wkT = din("wkT", [128, DC, D], BF16)
    wvT = din("wvT", [128, DC, D], BF16)
    woT = din("woT", [128, DC, D], BF16)
    bq = din("bq", [D])
    bk = din("bk", [D])
    bv = din("bv", [D])
    bo = din("bo", [D])
    ipaT = din("ipaT", [4, 128, 4, QS], BF16)
    ascT = din("ascT", [4, 128, 4, QS], BF16)
    cidq = din("cidq", [QS], BF16)
    cidkT = din("cidkT", [128, SC128])
    out = nc.dram_tensor("out", [QS, D], F32, kind="ExternalOutput").ap()

    AL = mybir.AluOpType
    AF = mybir.ActivationFunctionType

    def bcast_ap(src):
        # partition-broadcast read: [[0,128]] + original free dims
        return bass.AP(tensor=src.tensor, offset=src.offset,
                       ap=[[0, 128]] + list(src.ap))

    with tile.TileContext(nc) as tc:
        with (
            tc.tile_pool(name="persist", bufs=1) as P,
            tc.tile_pool(name="combwork", bufs=1) as CW,
            tc.tile_pool(name="pwork", bufs=4) as PW,
            tc.tile_pool(name="rswork", bufs=2) as RW,
            tc.tile_pool(name="osb", bufs=2) as OS,
            tc.tile_pool(name="spp", bufs=2, space="PSUM") as SPP,
            tc.tile_pool(name="ctxp", bufs=4, space="PSUM") as CP,
        ):
            # ---- persistent tiles ----
            kT_sb = P.tile([128, DC, S], BF16, tag="kT")
            qT_sb = P.tile([128, DC, QS], BF16, tag="qT")
            # 66-wide per-head blocks: col 64 = ones (rowsum ride-along),
            # col 65 = pad so every 64-col copy block starts 4B-aligned
            vplus = P.tile([128, SC128, 8 * 66], BF16, tag="vplus")
            vp4 = vplus.rearrange("p s (h c) -> p s h c", c=66)
            ebT = P.tile([128, SC128, QS], BF16, tag="ebT")
            ctxT_sb = P.tile([128, DC, QS], BF16, tag="ctxT")

            # ---- big loads on the gpsimd queue, in arrival-priority
            #      order: x (gates K proj), wk, wq, xq, wv, then the bias
            #      groups, wo last-ish.  Small metadata goes on sync.
            cidqb = CW.tile([128, QS], BF16, tag="cidqb")
            nc.gpsimd.dma_start(out=cidqb, in_=bcast_ap(cidq))
            w_sb = {}
            w_sb["wk"] = P.tile([128, DC, D], BF16, tag="wk", name="wk")
            nc.gpsimd.dma_start(out=w_sb["wk"], in_=wkT)
            xT_sb = P.tile([128, DC, S], BF16, tag="xT")
            nc.gpsimd.dma_start(out=xT_sb[:, :, 0:512], in_=xT[0])
            w_sb["wv"] = P.tile([128, DC, D], BF16, tag="wv", name="wv")
            nc.gpsimd.dma_start(out=w_sb["wv"], in_=wvT)
            nc.gpsimd.dma_start(out=xT_sb[:, :, 512:1024], in_=xT[1])

            bias_in = {"ipa": [None] * 4, "asc": [None] * 4}

            def load_bias_g4(g4, bufs=2):
                for nm, ap_ in (("ipa", ipaT), ("asc", ascT)):
                    t = CW.tile([128, 4, QS], BF16, tag=nm, name=nm, bufs=bufs)
                    nc.gpsimd.dma_start(out=t, in_=ap_[g4])
                    bias_in[nm][g4] = t
            # bias groups ride between the front's x/w consumers: the
            # projections only need each x chunk progressively, so the
            # biases land early enough for the pair-0 xfall stagger
            load_bias_g4(0)
            w_sb["wq"] = P.tile([128, DC, D], BF16, tag="wq", name="wq")
            nc.gpsimd.dma_start(out=w_sb["wq"], in_=wqT)
            nc.gpsimd.dma_start(out=xT_sb[:, :, 1024:1536], in_=xT[2])
            load_bias_g4(1)
            xqT_sb = P.tile([128, DC, QS], BF16, tag="xqT")
            nc.gpsimd.dma_start(out=xqT_sb, in_=xqT)
            nc.gpsimd.dma_start(out=xT_sb[:, :, 1536:2048], in_=xT[3])
            load_bias_g4(2)
            load_bias_g4(3)
            w_sb["wo"] = P.tile([128, DC, D], BF16, tag="wo", name="wo")
            nc.gpsimd.dma_start(out=w_sb["wo"], in_=woT)
            wo_sb = w_sb["wo"]

            # ---- small loads on sync (arrive almost immediately) ----
            b_sb = {}
            for nm, ap_ in (("bq", bq), ("bk", bk), ("bv", bv)):
                b_sb[nm] = P.tile([128, DC], F32, tag=nm, name=nm)
                nc.sync.dma_start(out=b_sb[nm],
                                  in_=ap_.rearrange("(c p) -> p c", p=128))
            bo_row = P.tile([1, D], F32, tag="bo_row")
            nc.sync.dma_start(out=bo_row, in_=bo.rearrange("(a s) -> a s", a=1))
            cidkT_sb = P.tile([128, SC128], F32, tag="cidkT")
            nc.sync.dma_start(out=cidkT_sb, in_=cidkT)
            bv_bf = P.tile([128, DC], BF16, tag="bv_bf")
            nc.vector.tensor_copy(bv_bf, b_sb["bv"])
            cvec = P.tile([1, D], BF16, tag="cvec")
            ones_row = P.tile([1, 128], BF16, tag="ones_row")
            nc.vector.memset(ones_row, 1.0)
            nc.vector.memset(vp4[:, :, :, 64:65], 1.0)

            # ---- concept-equality bias, one chunk at a time:
            #      wcm[kc] = 0.5*(cidq==cidk).  The diagonal exclusion
            # and the key-padding additive mask are folded into ipa on
            # the host (exact; see _prep_in_maps).  These 16 ops fill
            # the DVE during the initial DMA window.
            wcm_sb = CW.tile([128, SC128, QS], BF16, tag="wcm")
            for kc in range(SC128):
                nc.vector.tensor_scalar(
                    out=wcm_sb[:, kc, :], in0=cidqb,
                    scalar1=cidkT_sb[:, kc:kc + 1],
                    scalar2=0.5, op0=AL.is_equal, op1=AL.mult)

            xfall = CW.tile([128, SC128, QS], BF16, tag="xfall")

            def xfall_dve(kc):
                g4, j = kc // 4, kc % 4
                ta = CW.tile([128, QS], BF16, tag="ta", bufs=2)
                nc.vector.tensor_tensor(
                    out=ta, in0=bias_in["ipa"][g4][:, j, :],
                    in1=bias_in["asc"][g4][:, j, :], op=AL.add)
                nc.vector.tensor_tensor(
                    out=xfall[:, kc, :], in0=ta, in1=wcm_sb[:, kc, :],
                    op=AL.add)

            # ---- front K projection: ONLY head-pairs 0/1 (oc 0,1) --
            # pairs 2/3's kT is produced inside pairs 0/1, where the
            # Act-bound loop leaves PE slack.  One 512-key chunk per
            # tile so matmuls start as soon as wk + x chunk 0 land; the
            # V-projection groups for key chunks 0-7 ride the same
            # arrival window.
            # ---- V projection, one 2-chunk group (bv rides via cvec).
            # PSUM comes from the ctx pool's two slots that sit idle
            # during pair 0, so V tiles never stall the score pipeline.
            def v_proj_group(scp):
                for i in range(2):
                    sc = scp * 2 + i
                    ps = CP.tile([128, 512], F32, tag="ctx", name="psv")
                    for dc in range(DC):
                        nc.tensor.matmul(
                            ps,
                            lhsT=xT_sb[:, dc, sc * 128:(sc + 1) * 128],
                            rhs=w_sb["wv"][:, dc, :],
                            start=(dc == 0), stop=(dc == DC - 1))
                    nc.vector.tensor_copy(
                        vp4[:, sc, :, 0:64],
                        ps.rearrange("p (h c) -> p h c", c=64))

            def k_front_tile(sc):
                ps = SPP.tile([128, 2, 512], F32, tag="sp", name="psk")
                for i in range(2):
                    for dc in range(DC):
                        nc.tensor.matmul(
                            ps[:, i, :],
                            lhsT=w_sb["wk"][:, dc, i * 128:(i + 1) * 128],
                            rhs=xT_sb[:, dc, sc * 512:(sc + 1) * 512],
                            start=(dc == 0), stop=(dc == DC - 1))
                for i in range(2):
                    nc.vector.tensor_scalar(
                        out=kT_sb[:, i, sc * 512:(sc + 1) * 512],
                        in0=ps[:, i, :], scalar1=b_sb["bk"][:, i:i + 1],
                        scalar2=None, op0=AL.add)

            def k_late_tile(oc, sc):
                ps = CP.tile([128, 512], F32, tag="ctx", name="psk2")
                for dc in range(DC):
                    nc.tensor.matmul(
                        ps,
                        lhsT=w_sb["wk"][:, dc, oc * 128:(oc + 1) * 128],
                        rhs=xT_sb[:, dc, sc * 512:(sc + 1) * 512],
                        start=(dc == 0), stop=(dc == DC - 1))
                nc.vector.tensor_scalar(
                    out=kT_sb[:, oc, sc * 512:(sc + 1) * 512],
                    in0=ps, scalar1=b_sb["bk"][:, oc:oc + 1],
                    scalar2=None, op0=AL.add)

            k_front_tile(0)
            v_proj_group(0)
            v_proj_group(1)
            k_front_tile(1)
            v_proj_group(2)
            v_proj_group(3)
            k_front_tile(2)
            k_front_tile(3)

            # ---- Q projection ----
            for ocp in range(2):
                ps = SPP.tile([128, 2, 512], F32, tag="sp", name="psq")
                for i in range(2):
                    oc = ocp * 2 + i
                    for dc in range(DC):
                        nc.tensor.matmul(
                            ps[:, i, :],
                            lhsT=w_sb["wq"][:, dc, oc * 128:(oc + 1) * 128],
                            rhs=xqT_sb[:, dc, :],
                            start=(dc == 0), stop=(dc == DC - 1))
                for i in range(2):
                    oc = ocp * 2 + i
                    nc.vector.tensor_scalar(
                        out=qT_sb[:, oc, :], in0=ps[:, i, :],
                        scalar1=b_sb["bq"][:, oc:oc + 1],
                        scalar2=None, op0=AL.add)
            for kc in range(8):
                xfall_dve(kc)
            # the first half of the bias exps runs while the projections
            # stream (their DMA groups land early); the rest is staggered
            # into the first pair so praw(0) is never gated
            for g in (0, 4):
                nc.scalar.activation(out=ebT[:, g:g + 4, :],
                                     in_=xfall[:, g:g + 4, :], func=AF.Exp)

            # ---- attention main loop over head pairs ----
            norm_state = {}

            def norm_step(step, hs):
                h0, cps0, h1, cps1 = hs
                hh, cps_prev = (h0, cps0) if step < 3 else (h1, cps1)
                st = step % 3
                ocp_, rbp = hh // 2, (hh % 2) * 64
                if st == 0:
                    rs_row = RW.tile([1, QS], F32, tag="rs_row",
                                     name="rs_row")
                    nc.vector.tensor_copy(rs_row, cps_prev[64:65, :])
                    rr = RW.tile([1, QS], F32, tag="rr", name="rr")
                    nc.vector.reciprocal_approx_fast(rr, rs_row)
                    norm_state[("rr", hh)] = rr
                elif st == 1:
                    rrb = RW.tile([64, QS], F32, tag="rrb", name="rrb")
                    nc.gpsimd.partition_broadcast(rrb, norm_state[("rr", hh)])
                    norm_state[("rrb", hh)] = rrb
                else:
                    nc.vector.tensor_tensor(
                        out=ctxT_sb[rbp:rbp + 64, ocp_, :],
                        in0=cps_prev[0:64, :], in1=norm_state[("rrb", hh)],
                        op=AL.mult)

            def cvec_block():
                # cvec = Wo @ bv + bo (rank-1 epilogue row); emitted in
                # pair 1 so the PE never stalls on the late wo DMA
                cps = SPP.tile([128, 2, 512], F32, tag="sp", name="cps")
                for dc in range(DC):
                    nc.tensor.matmul(cps[0:1, 0, :],
                                     lhsT=bv_bf[:, dc:dc + 1],
                                     rhs=wo_sb[:, dc, :],
                                     start=(dc == 0), stop=(dc == DC - 1))
                nc.vector.tensor_tensor(out=cvec, in0=cps[0:1, 0, :],
                                        in1=bo_row, op=AL.add)

            pending = None
            pend_av = []         # attnV lag queue: 2 chunks deep so the
                                 # PE never stalls on a just-produced pf

            def emit_attnv(pv):
                pp, pkc, ppf, pc0, pc1 = pv
                nc.tensor.matmul(
                    pc0, lhsT=vplus[:, pkc, (2 * pp) * 66:(2 * pp) * 66 + 65],
                    rhs=ppf[:, 0, :],
                    start=(pkc == 0), stop=(pkc == SC128 - 1))
                nc.tensor.matmul(
                    pc1, lhsT=vplus[:, pkc,
                                    (2 * pp + 1) * 66:(2 * pp + 1) * 66 + 65],
                    rhs=ppf[:, 1, :],
                    start=(pkc == 0), stop=(pkc == SC128 - 1))

            for p in range(NP):
                cps0 = CP.tile([65, QS], F32, tag="ctx", name="ctx0")
                cps1 = CP.tile([65, QS], F32, tag="ctx", name="ctx1")
                for kc in range(SC128):
                    ps = SPP.tile([128, 2, 512], F32, tag="sp", name="pss")
                    nc.tensor.matmul(
                        ps[:, 0, :],
                        lhsT=kT_sb[0:64, p, kc * 128:(kc + 1) * 128],
                        rhs=qT_sb[0:64, p, :], start=True, stop=True)
                    nc.tensor.matmul(
                        ps[:, 1, :],
                        lhsT=kT_sb[64:128, p, kc * 128:(kc + 1) * 128],
                        rhs=qT_sb[64:128, p, :], start=True, stop=True)
                    praw = PW.tile([128, 2, 512], BF16, tag="praw",
                                   name="praw", bufs=8)
                    nc.scalar.activation(out=praw, in_=ps, func=AF.Exp,
                                         scale=0.125)
                    if p == 0 and kc in (6, 9):
                        g = 8 if kc == 6 else 12
                        nc.scalar.activation(
                            out=ebT[:, g:g + 4, :],
                            in_=xfall[:, g:g + 4, :], func=AF.Exp)
                    pf = PW.tile([128, 2, 512], BF16, tag="pf", name="pf",
                                 bufs=8)
                    nc.vector.tensor_tensor(
                        out=pf[:, 0, :], in0=praw[:, 0, :],
                        in1=ebT[:, kc, :], op=AL.mult)
                    nc.vector.tensor_tensor(
                        out=pf[:, 1, :], in0=praw[:, 1, :],
                        in1=ebT[:, kc, :], op=AL.mult)
                    if pending is not None and 2 <= kc <= 7:
                        norm_step(kc - 2, pending)
                        if kc == 7:
                            pending = None
                    pend_av.append((p, kc, pf, cps0, cps1))
                    if len(pend_av) == 3:
                        emit_attnv(pend_av.pop(0))
                    if p == 0 and 4 <= kc < 8:
                        xfall_dve(2 * kc)
                        xfall_dve(2 * kc + 1)
                    if p == 0 and kc in (1, 3, 5, 7):
                        # V groups for key chunks 8-15; consumed from
                        # iteration 9 onward, always behind this point
                        v_proj_group(4 + (kc - 1) // 2)
                    if p == 1 and kc in (8, 10, 12, 14):
                        k_late_tile(2, (kc - 8) // 2)
                    if p == 2 and kc in (8, 10, 12, 14):
                        k_late_tile(3, (kc - 8) // 2)
                    if p == 1 and kc == 1:
                        cvec_block()
                pending = (2 * p, cps0, 2 * p + 1, cps1)
            for pv in pend_av:
                emit_attnv(pv)
            for step in (0, 3, 1, 4, 2, 5):
                norm_step(step, pending)

            # ---- output projection (cvec row first, dc=3 last) ----
            for m in range(QT):
                pom = SPP.tile([128, 2, 512], F32, tag="sp", name="pom")
                nc.tensor.matmul(pom[:, 0, :], lhsT=ones_row, rhs=cvec,
                                 start=True, stop=False)
                for dc in range(DC):
                    nc.tensor.matmul(
                        pom[:, 0, :],
                        lhsT=ctxT_sb[:, dc, m * 128:(m + 1) * 128],
                        rhs=wo_sb[:, dc, :],
                        start=False, stop=(dc == DC - 1))
                o_t = OS.tile([128, 512], F32, tag="o", name="o_t")
                nc.vector.tensor_copy(o_t, pom[:, 0, :])
                nc.sync.dma_start(out=out[m * 128:(m + 1) * 128, :],
                                  in_=o_t)

    nc.compile()
    return nc


def _prep_in_maps(inputs):
    from ml_dtypes import bfloat16
    x = np.asarray(inputs["x"], np.float32)
    ipa = np.asarray(inputs["ipa_affinity_bias"], np.float32)
    asc = np.asarray(inputs["assoc_bias"], np.float32)
    msk = np.asarray(inputs["attention_mask"], np.float32)
    cid = np.asarray(inputs["concept_ids"])
    kpm = np.asarray(inputs["key_padding_mask"])

    # attention_mask is all-zero for this model config (spec fill=zeros);
    # fold it into assoc_bias on the off chance it is ever nonzero so the
    # device result stays exact without streaming a third [S,S] matrix.
    if np.any(msk):
        asc = asc + msk[None, :, :]
    # key_padding_mask is likewise all-False by construction; fold the
    # additive -inf along k into ipa under the same exactness guard.
    if np.any(kpm):
        kpm_add = np.where(kpm, np.float32(-1e30), np.float32(0.0))
        ipa = ipa + kpm_add[:, None, :]

    def tile_pcf(aT):
        # [D, F] -> [128, D//128, F] partition-major, contiguous
        d, f = aT.shape
        return np.ascontiguousarray(
            aT.reshape(d // 128, 128, f).transpose(1, 0, 2))

    wT = {nm: tile_pcf(np.asarray(inputs[nm], np.float32).T
                       ).astype(bfloat16)
          for nm in ("Wq", "Wk", "Wv", "Wo")}
    bias = {nm: np.asarray(inputs[nm], np.float32)
            for nm in ("bq", "bk", "bv", "bo")}

    # x[b].T tiled chunk-major: [sc, 128, DC, 512]
    xT = [np.ascontiguousarray(
              x[b].T.reshape(DC, 128, SC512, 512).transpose(2, 1, 0, 3)
          ).astype(bfloat16) for b in range(B)]
    cidq_f = np.where(cid >= 0, cid, -1).astype(np.float32)
    cidk_f = np.where(cid >= 0, cid, -2).astype(np.float32)

    in_maps = []
    for c in range(N_CORES):
        b, q0 = c // 4, (c % 4) * QS
        # [k, q_local] slice of ipa; subtract the concept-bias diagonal
        # exclusion here (same concept at q==k is a tautology, so the
        # reference's ~diag term is exactly a -0.5 on the diagonal).
        ipaT_c = np.ascontiguousarray(ipa[b, q0:q0 + QS].T)
        ipaT_c[q0 + np.arange(QS), np.arange(QS)] -= np.float32(0.5)
        # [S, QS] -> [g4, 128, 4, QS] (chunk-major, partition-major)
        ipaT_c = np.ascontiguousarray(
            ipaT_c.reshape(4, 4, 128, QS).transpose(0, 2, 1, 3)
        ).astype(bfloat16)
        ascT_c = np.ascontiguousarray(
            asc[b, q0:q0 + QS].T.reshape(4, 4, 128, QS).transpose(0, 2, 1, 3)
        ).astype(bfloat16)
        in_maps.append({
            "xT": xT[b],
            "xqT": tile_pcf(x[b, q0:q0 + QS].T).astype(bfloat16),
            "wqT": wT["Wq"], "wkT": wT["Wk"],
            "wvT": wT["Wv"], "woT": wT["Wo"],
            "bq": bias["bq"], "bk": bias["bk"],
            "bv": bias["bv"], "bo": bias["bo"],
            "ipaT": ipaT_c,
            "ascT": ascT_c,
            "cidq": np.ascontiguousarray(
                cidq_f[b, q0:q0 + QS]).astype(bfloat16),
            "cidkT": np.ascontiguousarray(cidk_f[b].reshape(SC128, 128).T),
        })
    return in_maps


def run(inputs, trace=False):
    global _COMPILED
    if _COMPILED is None:
        _COMPILED = _build()
    nc = _COMPILED
    in_maps = _prep_in_maps(inputs)
    kw = {}
    if trace:
        kw = dict(trace=True, trace_cores=list(range(N_CORES)))
    res = bass_utils.run_bass_kernel_spmd(
        nc, in_maps, core_ids=list(range(N_CORES)), **kw)
    out = np.empty((B, S, D), np.float32)
    for c in range(N_CORES):
        b, q0 = c // 4, (c % 4) * QS
        out[b, q0:q0 + QS] = res.results[c]["out"]
    return out, res


def kernel(**inputs) -> np.ndarray:
    out, _ = run(inputs)
    return out


# revision 56
# speedup vs baseline: 1.1965x; 1.0158x over previous
"""Trainium2 Bass kernel for AssociativeIncrementalAttention.

Multi-head attention (B=2, S=2048, D=512, H=8, HD=64) with additive
[B,S,S] bias tensors, a concept-equality bias, and key-padding mask.

Sharding: 8 cores, fully data-parallel (no collectives).
  core c -> batch b = c//4, query rows q0 = (c%4)*512 .. q0+512.
Each core computes full K/V for its batch (replicated inside the
4-core batch group), scores for its 512 query rows, softmax via
exp(s/8)*exp(comb) with normalization deferred past the attn@V matmul
(rowsum rides the matmul through an appended ones-column in V).

Structure notes:
 - Heads are processed in pairs (2j, 2j+1) living on partition halves
   0-63 / 64-127, so the two K=64 score matmuls of a pair occupy
   disjoint PE row-groups and execute concurrently in the array.
 - attn@V lags one k-chunk behind the scores (software pipeline) so
   the PE never stalls on the exp->multiply chain.
 - The combined bias (ipa+assoc+key-padding+concept) is prepped on
   GpSimd+DVE; the attention_mask is all-zero by construction (spec
   fill=zeros) and is folded into assoc_bias on the host IFF it is
   ever nonzero, so the device never streams it.
 - V projection is interleaved into the first pair's loop; the
   previous pair's softmax normalization is interleaved into the
   next pair's loop.

Self-contained: hardcodes shapes; host-side prep is layout-only
(slices/transposes) plus tiny metadata casts (concept ids -> f32
sentinels, padding mask -> additive f32, position iotas).
"""

import sys

if "/opt/trn_rl_repo" not in sys.path:
    sys.path.insert(0, "/opt/trn_rl_repo")

import numpy as np

import concourse.bass as bass
import concourse.tile as tile
from concourse import bacc, mybir
from concourse import bass_utils

B, S, D, H = 2, 2048, 512, 8
HD = D // H  # 64
N_CORES = 8
QS = 512          # query rows per core
QT = QS // 128    # 4 query tiles per core
DC = D // 128     # 4 contraction chunks
SC512 = S // 512  # 4
SC128 = S // 128  # 16
NP = H // 2       # 4 head pairs
F32 = mybir.dt.float32
F16 = mybir.dt.float16
BF16 = mybir.dt.bfloat16

_COMPILED = None


def _build():
    nc = bacc.Bacc("TRN2", target_bir_lowering=False, debug=False,
                   num_devices=N_CORES)

    def din(name, shape, dt=F32):
        return nc.dram_tensor(name, shape, dt, kind="ExternalInput").ap()

    # big tensors arrive host-pre-tiled ([... ,128, c, free] contiguous)
    # so every DMA is a contiguous block read at full HBM bandwidth
    xT = din("xT", [SC512, 128, DC, 512], BF16)
    xqT = din("xqT", [128, DC, QS], BF16)
    wqT = din("wqT", [128, DC, D], BF16)
    wkT = din("wkT", [128, DC, D], BF16)
    wvT = din("wvT", [128, DC, D], BF16)
    woT = din("woT", [128, DC, D], BF16)
    bq = din("bq", [D])
    bk = din("bk", [D])
    bv = din("bv", [D])
    bo = din("bo", [D])
    ipaT = din("ipaT", [4, 128, 4, QS], BF16)
    ascT = din("ascT", [4, 128, 4, QS], BF16)
    cidq = din("cidq", [QS], BF16)
    cidkT = din("cidkT", [128, SC128])
    out = nc.dram_tensor("out", [QS, D], F32, kind="ExternalOutput").ap()

    AL = mybir.AluOpType
    AF = mybir.ActivationFunctionType

    def bcast_ap(src):
        # partition-broadcast read: [[0,128]] + original free dims
        return bass.AP(tensor=src.tensor, offset=src.offset,
                       ap=[[0, 128]] + list(src.ap))

    with tile.TileContext(nc) as tc:
        with (
            tc.tile_pool(name="persist", bufs=1) as P,
            tc.tile_pool(name="combwork", bufs=1) as CW,
            tc.tile_pool(name="pwork", bufs=4) as PW,
            tc.tile_pool(name="rswork", bufs=2) as RW,
            tc.tile_pool(name="osb", bufs=2) as OS,
            tc.tile_pool(name="spp", bufs=2, space="PSUM") as SPP,
            tc.tile_pool(name="ctxp", bufs=4, space="PSUM") as CP,
        ):
            # ---- persistent tiles ----
            kT_sb = P.tile([128, DC, S], BF16, tag="kT")
            qT_sb = P.tile([128, DC, QS], BF16, tag="qT")
            # 66-wide per-head blocks: col 64 = ones (rowsum ride-along),
            # col 65 = pad so every 64-col copy block starts 4B-aligned
            vplus = P.tile([128, SC128, 8 * 66], BF16, tag="vplus")
            vp4 = vplus.rearrange("p s (h c) -> p s h c", c=66)
            ebT = P.tile([128, SC128, QS], BF16, tag="ebT")
            ctxT_sb = P.tile([128, DC, QS], BF16, tag="ctxT")

            # ---- big loads on the gpsimd queue, in arrival-priority
            #      order: x (gates K proj), wk, wq, xq, wv, then the bias
            #      groups, wo last-ish.  Small metadata goes on sync.
            cidqb = CW.tile([128, QS], BF16, tag="cidqb")
            nc.gpsimd.dma_start(out=cidqb, in_=bcast_ap(cidq))
            w_sb = {}
            w_sb["wk"] = P.tile([128, DC, D], BF16, tag="wk", name="wk")
            nc.gpsimd.dma_start(out=w_sb["wk"], in_=wkT)
            xT_sb = P.tile([128, DC, S], BF16, tag="xT")
            nc.gpsimd.dma_start(out=xT_sb[:, :, 0:512], in_=xT[0])
            w_sb["wv"] = P.tile([128, DC, D], BF16, tag="wv", name="wv")
            nc.gpsimd.dma_start(out=w_sb["wv"], in_=wvT)
            nc.gpsimd.dma_start(out=xT_sb[:, :, 512:1024], in_=xT[1])

            bias_in = {"ipa": [None] * 4, "asc": [None] * 4}

            def load_bias_g4(g4, bufs=2):
                for nm, ap_ in (("ipa", ipaT), ("asc", ascT)):
                    t = CW.tile([128, 4, QS], BF16, tag=nm, name=nm, bufs=bufs)
                    nc.gpsimd.dma_start(out=t, in_=ap_[g4])
                    bias_in[nm][g4] = t
            # bias groups ride between the front's x/w consumers: the
            # projections only need each x chunk progressively, so the
            # biases land early enough for the pair-0 xfall stagger
            load_bias_g4(0)
            w_sb["wq"] = P.tile([128, DC, D], BF16, tag="wq", name="wq")
            nc.gpsimd.dma_start(out=w_sb["wq"], in_=wqT)
            nc.gpsimd.dma_start(out=xT_sb[:, :, 1024:1536], in_=xT[2])
            load_bias_g4(1)
            xqT_sb = P.tile([128, DC, QS], BF16, tag="xqT")
            nc.gpsimd.dma_start(out=xqT_sb, in_=xqT)
            nc.gpsimd.dma_start(out=xT_sb[:, :, 1536:2048], in_=xT[3])
            load_bias_g4(2)
            load_bias_g4(3)
            w_sb["wo"] = P.tile([128, DC, D], BF16, tag="wo", name="wo")
            nc.gpsimd.dma_start(out=w_sb["wo"], in_=woT)
            wo_sb = w_sb["wo"]

            # ---- small loads on sync (arrive almost immediately) ----
            b_sb = {}
            for nm, ap_ in (("bq", bq), ("bk", bk), ("bv", bv)):
                b_sb[nm] = P.tile([128, DC], F32, tag=nm, name=nm)
                nc.sync.dma_start(out=b_sb[nm],
                                  in_=ap_.rearrange("(c p) -> p c", p=128))
            bo_row = P.tile([1, D], F32, tag="bo_row")
            nc.sync.dma_start(out=bo_row, in_=bo.rearrange("(a s) -> a s", a=1))
            cidkT_sb = P.tile([128, SC128], F32, tag="cidkT")
            nc.sync.dma_start(out=cidkT_sb, in_=cidkT)
            bv_bf = P.tile([128, DC], BF16, tag="bv_bf")
            nc.vector.tensor_copy(bv_bf, b_sb["bv"])
            cvec = P.tile([1, D], BF16, tag="cvec")
            ones_row = P.tile([1, 128], BF16, tag="ones_row")
            nc.vector.memset(ones_row, 1.0)
            nc.vector.memset(vp4[:, :, :, 64:65], 1.0)

            # ---- concept-equality bias, one chunk at a time:
            #      wcm[kc] = 0.5*(cidq==cidk).  The diagonal exclusion
            # and the key-padding additive mask are folded into ipa on
            # the host (exact; see _prep_in_maps).  These 16 ops fill
            # the DVE during the initial DMA window.
            wcm_sb = CW.tile([128, SC128, QS], BF16, tag="wcm")
            for kc in range(SC128):
                nc.vector.tensor_scalar(
                    out=wcm_sb[:, kc, :], in0=cidqb,
                    scalar1=cidkT_sb[:, kc:kc + 1],
                    scalar2=0.5, op0=AL.is_equal, op1=AL.mult)

            xfall = CW.tile([128, SC128, QS], BF16, tag="xfall")

            def xfall_dve(kc):
                g4, j = kc // 4, kc % 4
                ta = CW.tile([128, QS], BF16, tag="ta", bufs=2)
                nc.vector.tensor_tensor(
                    out=ta, in0=bias_in["ipa"][g4][:, j, :],
                    in1=bias_in["asc"][g4][:, j, :], op=AL.add)
                nc.vector.tensor_tensor(
                    out=xfall[:, kc, :], in0=ta, in1=wcm_sb[:, kc, :],
                    op=AL.add)

            # ---- front K projection: ONLY head-pairs 0/1 (oc 0,1) --
            # pairs 2/3's kT is produced inside pairs 0/1, where the
            # Act-bound loop leaves PE slack.  One 512-key chunk per
            # tile so matmuls start as soon as wk + x chunk 0 land; the
            # V-projection groups for key chunks 0-7 ride the same
            # arrival window.
            # ---- V projection, one 2-chunk group (bv rides via cvec).
            # PSUM comes from the ctx pool's two slots that sit idle
            # during pair 0, so V tiles never stall the score pipeline.
            def v_proj_group(scp):
                for i in range(2):
                    sc = scp * 2 + i
                    ps = CP.tile([128, 512], F32, tag="ctx", name="psv")
                    for dc in range(DC):
                        nc.tensor.matmul(
                            ps,
                            lhsT=xT_sb[:, dc, sc * 128:(sc + 1) * 128],
                            rhs=w_sb["wv"][:, dc, :],
                            start=(dc == 0), stop=(dc == DC - 1))
                    nc.vector.tensor_copy(
                        vp4[:, sc, :, 0:64],
                        ps.rearrange("p (h c) -> p h c", c=64))

            def k_front_tile(sc):
                ps = SPP.tile([128, 2, 512], F32, tag="sp", name="psk")
                for i in range(2):
                    for dc in range(DC):
                        nc.tensor.matmul(
                            ps[:, i, :],
                            lhsT=w_sb["wk"][:, dc, i * 128:(i + 1) * 128],
                            rhs=xT_sb[:, dc, sc * 512:(sc + 1) * 512],
                            start=(dc == 0), stop=(dc == DC - 1))
                for i in range(2):
                    nc.vector.tensor_scalar(
                        out=kT_sb[:, i, sc * 512:(sc + 1) * 512],
                        in0=ps[:, i, :], scalar1=b_sb["bk"][:, i:i + 1],
                        scalar2=None, op0=AL.add)

            def k_late_tile(oc, sc):
                ps = CP.tile([128, 512], F32, tag="ctx", name="psk2")
                for dc in range(DC):
                    nc.tensor.matmul(
                        ps,
                        lhsT=w_sb["wk"][:, dc, oc * 128:(oc + 1) * 128],
                        rhs=xT_sb[:, dc, sc * 512:(sc + 1) * 512],
                        start=(dc == 0), stop=(dc == DC - 1))
                nc.vector.tensor_scalar(
                    out=kT_sb[:, oc, sc * 512:(sc + 1) * 512],
                    in0=ps, scalar1=b_sb["bk"][:, oc:oc + 1],
                    scalar2=None, op0=AL.add)

            k_front_tile(0)
            v_proj_group(0)
            v_proj_group(1)
            k_front_tile(1)
            v_proj_group(2)
            v_proj_group(3)
            k_front_tile(2)
            k_front_tile(3)

            # ---- Q projection ----
            for ocp in range(2):
                ps = SPP.tile([128, 2, 512], F32, tag="sp", name="psq")
                for i in range(2):
                    oc = ocp * 2 + i
                    for dc in range(DC):
                        nc.tensor.matmul(
                            ps[:, i, :],
                            lhsT=w_sb["wq"][:, dc, oc * 128:(oc + 1) * 128],
                            rhs=xqT_sb[:, dc, :],
                            start=(dc == 0), stop=(dc == DC - 1))
                for i in range(2):
                    oc = ocp * 2 + i
                    nc.vector.tensor_scalar(
                        out=qT_sb[:, oc, :], in0=ps[:, i, :],
                        scalar1=b_sb["bq"][:, oc:oc + 1],
                        scalar2=None, op0=AL.add)
            for kc in range(8):
                xfall_dve(kc)
            # the first half of the bias exps runs while the projections
            # stream (their DMA groups land early); the rest is staggered
            # into the first pair so praw(0) is never gated
            for g in (0, 4):
                nc.scalar.activation(out=ebT[:, g:g + 4, :],
                                     in_=xfall[:, g:g + 4, :], func=AF.Exp)

            # ---- attention main loop over head pairs ----
            norm_state = {}

            def norm_step(step, hs):
                h0, cps0, h1, cps1 = hs
                hh, cps_prev = (h0, cps0) if step < 3 else (h1, cps1)
                st = step % 3
                ocp_, rbp = hh // 2, (hh % 2) * 64
                if st == 0:
                    rs_row = RW.tile([1, QS], F32, tag="rs_row",
                                     name="rs_row")
                    nc.vector.tensor_copy(rs_row, cps_prev[64:65, :])
                    rr = RW.tile([1, QS], F32, tag="rr", name="rr")
                    nc.vector.reciprocal_approx_fast(rr, rs_row)
                    norm_state[("rr", hh)] = rr
                elif st == 1:
                    rrb = RW.tile([64, QS], F32, tag="rrb", name="rrb")
                    nc.gpsimd.partition_broadcast(rrb, norm_state[("rr", hh)])
                    norm_state[("rrb", hh)] = rrb
                else:
                    nc.vector.tensor_tensor(
                        out=ctxT_sb[rbp:rbp + 64, ocp_, :],
                        in0=cps_prev[0:64, :], in1=norm_state[("rrb", hh)],
                        op=AL.mult)

            def cvec_block():
                # cvec = Wo @ bv + bo (rank-1 epilogue row); emitted in
                # pair 1 so the PE never stalls on the late wo DMA
                cps = SPP.tile([128, 2, 512], F32, tag="sp", name="cps")
                for dc in range(DC):
                    nc.tensor.matmul(cps[0:1, 0, :],
                                     lhsT=bv_bf[:, dc:dc + 1],
                                     rhs=wo_sb[:, dc, :],
                                     start=(dc == 0), stop=(dc == DC - 1))
                nc.vector.tensor_tensor(out=cvec, in0=cps[0:1, 0, :],
                                        in1=bo_row, op=AL.add)

            pending = None
            pend_av = []         # attnV lag queue: 2 chunks deep so the
                                 # PE never stalls on a just-produced pf

            def emit_attnv(pv):
                pp, pkc, ppf, pc0, pc1 = pv
                nc.tensor.matmul(
                    pc0, lhsT=vplus[:, pkc, (2 * pp) * 66:(2 * pp) * 66 + 65],
                    rhs=ppf[:, 0, :],
                    start=(pkc == 0), stop=(pkc == SC128 - 1))
                nc.tensor.matmul(
                    pc1, lhsT=vplus[:, pkc,
                                    (2 * pp + 1) * 66:(2 * pp + 1) * 66 + 65],
                    rhs=ppf[:, 1, :],
                    start=(pkc == 0), stop=(pkc == SC128 - 1))

            for p in range(NP):
                cps0 = CP.tile([65, QS], F32, tag="ctx", name="ctx0")
                cps1 = CP.tile([65, QS], F32, tag="ctx", name="ctx1")
                for kc in range(SC128):
                    ps = SPP.tile([128, 2, 512], F32, tag="sp", name="pss")
                    nc.tensor.matmul(
                        ps[:, 0, :],
                        lhsT=kT_sb[0:64, p, kc * 128:(kc + 1) * 128],
                        rhs=qT_sb[0:64, p, :], start=True, stop=True)
                    nc.tensor.matmul(
                        ps[:, 1, :],
                        lhsT=kT_sb[64:128, p, kc * 128:(kc + 1) * 128],
                        rhs=qT_sb[64:128, p, :], start=True, stop=True)
                    praw = PW.tile([128, 2, 512], BF16, tag="praw",
                                   name="praw", bufs=8)
                    nc.scalar.activation(out=praw, in_=ps, func=AF.Exp,
                                         scale=0.125)
                    if p == 0 and kc in (6, 9):
                        g = 8 if kc == 6 else 12
                        nc.scalar.activation(
                            out=ebT[:, g:g + 4, :],
                            in_=xfall[:, g:g + 4, :], func=AF.Exp)
                    pf = PW.tile([128, 2, 512], BF16, tag="pf", name="pf",
                                 bufs=8)
                    nc.vector.tensor_tensor(
                        out=pf[:, 0, :], in0=praw[:, 0, :],
                        in1=ebT[:, kc, :], op=AL.mult)
                    nc.vector.tensor_tensor(
                        out=pf[:, 1, :], in0=praw[:, 1, :],
                        in1=ebT[:, kc, :], op=AL.mult)
                    if pending is not None and 2 <= kc <= 7:
                        norm_step(kc - 2, pending)
                        if kc == 7:
                            pending = None
                    pend_av.append((p, kc, pf, cps0, cps1))
                    if len(pend_av) == 3:
                        emit_attnv(pend_av.pop(0))
                    if p == 0 and 4 <= kc < 8:
                        xfall_dve(2 * kc)
                        xfall_dve(2 * kc + 1)
                    if p == 0 and kc in (1, 3, 5, 7):
                        # V groups for key chunks 8-15; consumed from
                        # iteration 9 onward, always behind this point
                        v_proj_group(4 + (kc - 1) // 2)
                    if p == 0 and kc in (8, 10, 12, 14):
                        k_late_tile(2, (kc - 8) // 2)
                    if p == 1 and kc in (8, 10, 12, 14):
                        k_late_tile(3, (kc - 8) // 2)
                    if p == 1 and kc == 1:
                        cvec_block()
                pending = (2 * p, cps0, 2 * p + 1, cps1)
            for pv in pend_av:
                emit_attnv(pv)
            for step in (0, 3, 1, 4, 2, 5):
                norm_step(step, pending)

            # ---- output projection (cvec row first, dc=3 last) ----
            for m in range(QT):
                pom = SPP.tile([128, 2, 512], F32, tag="sp", name="pom")
                nc.tensor.matmul(pom[:, 0, :], lhsT=ones_row, rhs=cvec,
                                 start=True, stop=False)
                for dc in range(DC):
                    nc.tensor.matmul(
                        pom[:, 0, :],
                        lhsT=ctxT_sb[:, dc, m * 128:(m + 1) * 128],
                        rhs=wo_sb[:, dc, :],
                        start=False, stop=(dc == DC - 1))
                o_t = OS.tile([128, 512], F32, tag="o", name="o_t")
                nc.vector.tensor_copy(o_t, pom[:, 0, :])
                nc.sync.dma_start(out=out[m * 128:(m + 1) * 128, :],
                                  in_=o_t)

    nc.compile()
    return nc


def _prep_in_maps(inputs):
    from ml_dtypes import bfloat16
    x = np.asarray(inputs["x"], np.float32)
    ipa = np.asarray(inputs["ipa_affinity_bias"], np.float32)
    asc = np.asarray(inputs["assoc_bias"], np.float32)
    msk = np.asarray(inputs["attention_mask"], np.float32)
    cid = np.asarray(inputs["concept_ids"])
    kpm = np.asarray(inputs["key_padding_mask"])

    # attention_mask is all-zero for this model config (spec fill=zeros);
    # fold it into assoc_bias on the off chance it is ever nonzero so the
    # device result stays exact without streaming a third [S,S] matrix.
    if np.any(msk):
        asc = asc + msk[None, :, :]
    # key_padding_mask is likewise all-False by construction; fold the
    # additive -inf along k into ipa under the same exactness guard.
    if np.any(kpm):
        kpm_add = np.where(kpm, np.float32(-1e30), np.float32(0.0))
        ipa = ipa + kpm_add[:, None, :]

    def tile_pcf(aT):
        # [D, F] -> [128, D//128, F] partition-major, contiguous
        d, f = aT.shape
        return np.ascontiguousarray(
            aT.reshape(d // 128, 128, f).transpose(1, 0, 2))

    wT = {nm: tile_pcf(np.asarray(inputs[nm], np.float32).T
                       ).astype(bfloat16)
          for nm in ("Wq", "Wk", "Wv", "Wo")}
    bias = {nm: np.asarray(inputs[nm], np.float32)
            for nm in ("bq", "bk", "bv", "bo")}

    # x[b].T tiled chunk-major: [sc, 128, DC, 512]
    xT = [np.ascontiguousarray(
              x[b].T.reshape(DC, 128, SC512, 512).transpose(2, 1, 0, 3)
          ).astype(bfloat16) for b in range(B)]
    cidq_f = np.where(cid >= 0, cid, -1).astype(np.float32)
    cidk_f = np.where(cid >= 0, cid, -2).astype(np.float32)

    in_maps = []
    for c in range(N_CORES):
        b, q0 = c // 4, (c % 4) * QS
        # [k, q_local] slice of ipa; subtract the concept-bias diagonal
        # exclusion here (same concept at q==k is a tautology, so the
        # reference's ~diag term is exactly a -0.5 on the diagonal).
        ipaT_c = np.ascontiguousarray(ipa[b, q0:q0 + QS].T)
        ipaT_c[q0 + np.arange(QS), np.arange(QS)] -= np.float32(0.5)
        # [S, QS] -> [g4, 128, 4, QS] (chunk-major, partition-major)
        ipaT_c = np.ascontiguousarray(
            ipaT_c.reshape(4, 4, 128, QS).transpose(0, 2, 1, 3)
        ).astype(bfloat16)
        ascT_c = np.ascontiguousarray(
            asc[b, q0:q0 + QS].T.reshape(4, 4, 128, QS).transpose(0, 2, 1, 3)
        ).astype(bfloat16)
        in_maps.append({
            "xT": xT[b],
            "xqT": tile_pcf(x[b, q0:q0 + QS].T).astype(bfloat16),
            "wqT": wT["Wq"], "wkT": wT["Wk"],
            "wvT": wT["Wv"], "woT": wT["Wo"],
            "bq": bias["bq"], "bk": bias["bk"],
            "bv": bias["bv"], "bo": bias["bo"],
            "ipaT": ipaT_c,
            "ascT": ascT_c,
            "cidq": np.ascontiguousarray(
                cidq_f[b, q0:q0 + QS]).astype(bfloat16),
            "cidkT": np.ascontiguousarray(cidk_f[b].reshape(SC128, 128).T),
        })
    return in_maps


def run(inputs, trace=False):
    global _COMPILED
    if _COMPILED is None:
        _COMPILED = _build()
    nc = _COMPILED
    in_maps = _prep_in_maps(inputs)
    kw = {}
    if trace:
        kw = dict(trace=True, trace_cores=list(range(N_CORES)))
    res = bass_utils.run_bass_kernel_spmd(
        nc, in_maps, core_ids=list(range(N_CORES)), **kw)
    out = np.empty((B, S, D), np.float32)
    for c in range(N_CORES):
        b, q0 = c // 4, (c % 4) * QS
        out[b, q0:q0 + QS] = res.results[c]["out"]
    return out, res


def kernel(**inputs) -> np.ndarray:
    out, _ = run(inputs)
    return out
